# revision 24
# baseline (speedup 1.0000x reference)
"""AttnNet kernel for Trainium2: attn = softmax(einsum("bsh,bh->bs", facts, questions))[:, None, :].

Full shapes: questions [64, 4096] f32, facts [64, 512, 4096] f32 -> out [64, 1, 512] f32.
Data-parallel over batch: 8 batches per NeuronCore x 8 cores, no collectives.

Per-core dataflow (B_LOC=8, S=512, H=4096):
  - facts streamed as 32 contiguous [128(s), 4096(h)] tiles (2 MiB each) on the
    sync HWDGE ring only (a second ring slows every descriptor -- measured).
  - q[b] is broadcast across partitions by the PE array (ones[1,128].T @
    q_row[1,512] per PSUM bank) straight into PSUM, split in two half-tiles
    [128, 2048] (h-halves). DVE reads it from there. No SBUF write traffic for
    the broadcast at all: gpsimd partition_broadcast's 2 MiB/batch of SBUF
    writes were stealing ~5.7 us from every batch-boundary DVE op, and a DMA
    replicate instead costs ~78 us of DMA-engine time per batch.
  - Fused DVE affine_mul_reduce per half-tile (custom DVE op; the native
    TensorTensorReduce opcode crashes on HW): (facts*q) row-sum -> E2 column,
    product written in place over the facts half-tile. ACT and gpsimd do no
    per-tile work.
  - h0/h1 interleaved per chunk so the PE rebroadcast of the next batch's q
    half overlaps the current batch's other-half affines.
  - Epilogue: E = E2 even+odd cols, DVE 32x32 block transposes E [128,32] ->
    [32,128], regroup to [8, 512] via SBUF->SBUF DMA, then softmax: -max (DVE),
    fused exp+sum (ACT), reciprocal + scale (DVE), DMA out.
"""

import numpy as np

B, S, H = 64, 512, 4096
N_CORES = 8
B_LOC = B // N_CORES  # 8
P = 128
SC = S // P  # 4 s-chunks per batch
HH = H // 2  # h half
BANK = 512  # PSUM bank width in f32

_CACHE = {}


def _build_bass():
    import concourse.bacc as bacc
    import concourse.mybir as mybir
    import concourse.tile as tile

    f32 = mybir.dt.float32

    nc = bacc.Bacc("TRN2", target_bir_lowering=False, debug=False)
    facts = nc.dram_tensor("facts", [B_LOC, S, H], f32, kind="ExternalInput").ap()
    questions = nc.dram_tensor("questions", [B_LOC, H], f32, kind="ExternalInput").ap()
    attn = nc.dram_tensor("attn", [B_LOC, S], f32, kind="ExternalOutput").ap()

    with tile.TileContext(nc) as tc:
        with (
            tc.tile_pool(name="consts", bufs=1) as consts,
            tc.tile_pool(name="fpool", bufs=8) as fpool,
            tc.tile_pool(name="qrow", bufs=2) as qrow,
            tc.tile_pool(name="qsb", bufs=2) as qsb,
            tc.tile_pool(name="pq", bufs=1, space="PSUM") as pqpool,
        ):
            ones = consts.tile([1, P], f32)
            nc.gpsimd.memset(ones[:], 1.0)
            # dependency-free first op on DVE: pays the engine's one-time
            # kernel-scope entry cost (~6 us) before the real inputs land
            warm = consts.tile([1, 4], f32)
            nc.vector.memset(warm[:], 0.0)

            # half-chunk energies: E2[:, 2*col+j] = sum over h-half j
            E2 = consts.tile([P, B_LOC * SC * 2], f32)
            # q's h0 half lives in PSUM, double buffered by batch parity: the
            # PE has the whole previous batch (~20 us) to broadcast the next
            # q's h0 into the idle buffer, so DVE never waits on it. (PSUM is
            # 16 KiB/partition: holding all of q double-buffered won't fit,
            # and single-buffered the PE swap stalls DVE at batch boundaries.)
            ph0_a = pqpool.tile([P, HH], f32)
            ph0_b = pqpool.tile([P, HH], f32)
            ph0 = [ph0_a, ph0_b]
            # dependency-free first matmul: starts the PE's p-state clock ramp
            # before the real q broadcast needs it (the target is overwritten
            # by that broadcast right after)
            nc.tensor.matmul(
                ph0_a[:, 0:4], ones[:], ones[:, 0:4], start=True, stop=True
            )

            for b in range(B_LOC):
                q_row = qrow.tile([1, H], f32)
                # batch 0's q_row rides first on the sync ring (one 16 KiB
                # descriptor, lands ~9 us); the scalar ring's DGE takes ~14 us
                # to deliver it and would gate the whole PE->affine ramp
                qring = nc.sync if b == 0 else nc.scalar
                qring.dma_start(out=q_row[:], in_=questions[b : b + 1, :])
                # PE broadcast of q's h0 into this batch's PSUM parity buffer
                qp = ph0[b % 2]
                for n in range(HH // BANK):
                    nc.tensor.matmul(
                        qp[:, n * BANK : (n + 1) * BANK],
                        ones[:],
                        q_row[:, n * BANK : (n + 1) * BANK],
                        start=True,
                        stop=True,
                    )
                # q's h1 half goes to SBUF: batch 0 via DMA replicate (the
                # gpsimd Q7 cores take ~12 us to warm up), later batches via
                # gpsimd partition_broadcast (half volume halves the SBUF
                # write contention it inflicts on concurrent DVE ops)
                q_s1 = qsb.tile([P, HH], f32)
                if b == 0:
                    nc.scalar.dma_start(
                        out=q_s1[:],
                        in_=questions[0:1, HH:].partition_broadcast(P),
                    )
                else:
                    nc.gpsimd.partition_broadcast(q_s1[:], q_row[:, HH:])

                for c in range(SC):
                    ftile = fpool.tile([P, H], f32)
                    nc.sync.dma_start(
                        out=ftile[:], in_=facts[b, c * P : (c + 1) * P, :]
                    )
                    col = b * SC + c
                    for j, qsrc in ((0, qp), (1, q_s1)):
                        nc.vector.affine_mul_reduce(
                            out=ftile[:, j * HH : (j + 1) * HH],
                            accum_out=E2[:, 2 * col + j : 2 * col + j + 1],
                            in0=ftile[:, j * HH : (j + 1) * HH],
                            in1=qsrc[:],
                            scale=1.0,
                            bias=0.0,
                        )

            # --- softmax epilogue (ACT only does the exp) ---
            E = consts.tile([P, B_LOC * SC], f32)
            nc.vector.tensor_add(out=E[:], in0=E2[:, 0::2], in1=E2[:, 1::2])
            # DVE 32x32 block transposes: E [128, 32] -> e_t [32, 128]
            e_t = consts.tile([B_LOC * SC, P], f32)
            for r in range(4):
                nc.vector.transpose(
                    e_t[:, 32 * r : 32 * (r + 1)], E[32 * r : 32 * (r + 1), :]
                )
            # regroup [32, 128] (p = b*4+c) -> [8, 512]
            e_rows = consts.tile([B_LOC, S], f32)
            nc.sync.dma_start(
                out=e_rows[:].rearrange("b (c i) -> b c i", i=P), in_=e_t[:]
            )

            neg_max = consts.tile([B_LOC, 1], f32)
            nc.vector.reduce_max(
                neg_max[:], e_rows[:], axis=mybir.AxisListType.X, negate=True
            )

            p_exp = consts.tile([B_LOC, S], f32)
            den = consts.tile([B_LOC, 1], f32)
            nc.scalar.activation(
                p_exp[:],
                e_rows[:],
                mybir.ActivationFunctionType.Exp,
                bias=neg_max[:],
                scale=1.0,
                accum_out=den[:],
            )

            recip = consts.tile([B_LOC, 1], f32)
            nc.vector.reciprocal(recip[:], den[:])

            a_t = consts.tile([B_LOC, S], f32)
            nc.vector.tensor_scalar_mul(a_t[:], p_exp[:], recip[:])

            nc.sync.dma_start(out=attn, in_=a_t[:])

    nc.compile()
    return nc


def _get_nc():
    if "nc" not in _CACHE:
        _CACHE["nc"] = _build_bass()
    return _CACHE["nc"]


def _shard_inputs(questions, facts):
    questions = np.ascontiguousarray(np.asarray(questions), dtype=np.float32)
    facts = np.ascontiguousarray(np.asarray(facts), dtype=np.float32)
    in_maps = []
    for i in range(N_CORES):
        sl = slice(i * B_LOC, (i + 1) * B_LOC)
        in_maps.append(
            {
                "facts": np.ascontiguousarray(facts[sl]),
                "questions": np.ascontiguousarray(questions[sl]),
            }
        )
    return in_maps


def _run(questions, facts, **run_kwargs):
    from concourse.bass_utils import run_bass_kernel_spmd

    nc = _get_nc()
    in_maps = _shard_inputs(questions, facts)
    res = run_bass_kernel_spmd(nc, in_maps, core_ids=list(range(N_CORES)), **run_kwargs)
    out = np.stack([np.asarray(res.results[i]["attn"]) for i in range(N_CORES)])
    return out.reshape(B, S)[:, None, :].astype(np.float32), res


def kernel(questions, facts):
    out, _ = _run(questions, facts)
    return out
